# revision 66
# baseline (speedup 1.0000x reference)
"""Trainium2 Bass kernel for CrossBandWindowAttention.

Reference computation (per window item b of B_=2048):
    q = (x @ Wq + bq) * scale      -> (64, 96), 6 heads x 16
    k = cross_x @ Wk + bk          -> (64, 96)
    v = cross_x @ Wv + bv          -> (64, 384), 6 heads x 64
    L_h = q_h k_h^T + rpb_bias_h (+ mask_w)
    A = softmax(L, axis=-1)
    out = (concat_h A_h v_h) @ Wp + bp

Sharding: data-parallel over b_ across 8 cores (256 windows each).
Weights / bias table replicated; rpi+rpb_table folded on host into a
(128, 384) additive bias tile in TRANSPOSED layout (rows = two windows'
64 m-tokens, cols = (h, n)).

Per-core design (all PE operands f16 -> full-rate streaming + FWL).
NOTE: this runtime crashes on >2 back-to-back matmuls whose
tile_position row strips differ, so everything uses row-0 tiles with at
most 2-way column packing (the baseline-proven constructs). Instruction
count is cut by K-dim stacking with zero-padded moving operands instead:
  - x/cross_x loaded natural (tokens on partitions), cast to f16 on
    DVE/ACT, then PE-transposed (f16, 1 cyc/row) to xt/cxt
    (channels on partitions, tokens free).
  - QK computes the TRANSPOSED logits L^T[m, (h, n)] for one window in
    ONE matmul: lhsT = kT natural [96=(h,d), 64 m-toks] (all heads
    stacked along K), rhs = th_q6 [96, (h, n)] where block h holds q_h
    at rows 16h..16h+16 and ZEROS elsewhere -- the zeros kill cross-head
    terms, and PE time only depends on streamed columns (384).
    th_q6 is filled by 6 SBUF-SBUF DMAs per group into persistent
    zero-initialized double buffers.
  - softmax: exp on ACT; multiply by exp(bias^T); per-(h, n) sums via a
    ones-matmul (lhsT = block-indicator [128, 2]); reciprocal on DVE;
    broadcast back over partitions via a K=2 matmul; normalize on DVE
    into een2 [128, (h, s, n)] with the same zero-block trick.
    No probability transposes, no max-subtraction (logits are O(1)).
  - AV: one matmul per head (K=128 = both windows' m stacked, v stays
    pair-natural -- no splits), N=128 = (s, n), col-packed in head
    pairs, into the proj-lhsT tile pl[(h%2)*64+vd, (Ci, s, n)].
  - V / final projections: 3 accumulating matmuls each per pair.
PSUM plan (8 banks): transposes+qkT pool 2, logits^T+recb pool 2,
sums 1, v/av/final rotating pool 3.
"""

import os
from contextlib import ExitStack

import numpy as np

import concourse.bass as bass
import concourse.mybir as mybir
import concourse.tile as tile
from concourse import bacc
from concourse.bass_utils import run_bass_kernel_spmd
from concourse.masks import make_identity

F32 = mybir.dt.float32
F32R = mybir.dt.float32r
F16 = mybir.dt.float16

DIM = 96
HEADS = 6
HD = 16  # head dim for q/k
VD = 64  # head dim for v
N = 64  # tokens per window
C = 384
NCORES = 8
B_TOTAL = 2048
NW_CORE = B_TOTAL // NCORES  # 256 windows per core
GRP = 8  # windows per group (512 tokens)
TOK_G = GRP * N  # 512


def _build(nw, use_mask, use_bias):
    """Build the per-core Bass module for `nw` windows."""
    nc = bacc.Bacc("TRN2", target_bir_lowering=False, debug=False)

    d_x = nc.dram_tensor("x", [nw, N, C], F32, kind="ExternalInput").ap()
    d_cx = nc.dram_tensor("cx", [nw, N, C], F32, kind="ExternalInput").ap()
    d_wq = nc.dram_tensor("wq", [C, DIM], F16, kind="ExternalInput").ap()
    d_wk = nc.dram_tensor("wk", [C, DIM], F16, kind="ExternalInput").ap()
    d_wv = nc.dram_tensor("wv", [C, C], F16, kind="ExternalInput").ap()
    d_wp = nc.dram_tensor("wp", [C, C], F16, kind="ExternalInput").ap()
    d_bias2 = nc.dram_tensor("bias2t", [128, C], F32, kind="ExternalInput").ap()
    d_selr = nc.dram_tensor("selr", [128, 128], F16, kind="ExternalInput").ap()
    if use_bias:
        d_bq = nc.dram_tensor("bq_c", [DIM, 1], F32, kind="ExternalInput").ap()
        d_bk = nc.dram_tensor("bk_c", [DIM, 1], F32, kind="ExternalInput").ap()
        d_bv2 = nc.dram_tensor("bv2", [128, C], F32, kind="ExternalInput").ap()
        d_bp2 = nc.dram_tensor("bp2", [128, C], F32, kind="ExternalInput").ap()
    if use_mask:
        d_mask2 = nc.dram_tensor(
            "mask2t", [nw // 2, 128, C], F32, kind="ExternalInput"
        ).ap()
    d_y = nc.dram_tensor("y", [nw, N, C], F32, kind="ExternalOutput").ap()

    x_flat = d_x.rearrange("w n c -> (w n) c")
    cx_flat = d_cx.rearrange("w n c -> (w n) c")
    y_flat = d_y.rearrange("w n c -> (w n) c")

    n_grp = nw // GRP

    with tile.TileContext(nc) as tc, ExitStack() as ctx:
        const = ctx.enter_context(tc.tile_pool(name="const", bufs=1))
        p_nat = ctx.enter_context(tc.tile_pool(name="p_nat", bufs=2))
        p_n16 = ctx.enter_context(tc.tile_pool(name="p_n16", bufs=2))
        p_xt = ctx.enter_context(tc.tile_pool(name="p_xt", bufs=2))
        p_qk = ctx.enter_context(tc.tile_pool(name="p_qk", bufs=2))
        p_v = ctx.enter_context(tc.tile_pool(name="p_v", bufs=2))
        p_sm = ctx.enter_context(tc.tile_pool(name="p_sm", bufs=2))
        p_pl = ctx.enter_context(tc.tile_pool(name="p_pl", bufs=2))
        p_out = ctx.enter_context(tc.tile_pool(name="p_out", bufs=2))
        # PSUM: 8 banks total.
        ps_g = ctx.enter_context(tc.tile_pool(name="ps_g", bufs=2, space="PSUM"))
        ps_l = ctx.enter_context(tc.tile_pool(name="ps_l", bufs=2, space="PSUM"))
        ps_vpf = ctx.enter_context(tc.tile_pool(name="ps_vpf", bufs=4, space="PSUM"))

        # ---- constants in SBUF ----
        identc = const.tile([128, 128], F16, name="identc")
        make_identity(nc, identc[:])

        wq_sb = const.tile([128, 3, DIM], F16, name="wq_sb")
        wk_sb = const.tile([128, 3, DIM], F16, name="wk_sb")
        wv_sb = const.tile([128, 3, C], F16, name="wv_sb")
        wp_sb = const.tile([128, 3, C], F16, name="wp_sb")
        bias2_sb = const.tile([128, C], F32, name="bias2_sb")
        selr_sb = const.tile([128, 128], F16, name="selr_sb")
        for Ci in range(3):
            sl = slice(128 * Ci, 128 * Ci + 128)
            nc.sync.dma_start(wq_sb[:, Ci], d_wq[sl, :])
            nc.sync.dma_start(wk_sb[:, Ci], d_wk[sl, :])
            nc.sync.dma_start(wv_sb[:, Ci], d_wv[sl, :])
            nc.sync.dma_start(wp_sb[:, Ci], d_wp[sl, :])
        nc.sync.dma_start(bias2_sb[:], d_bias2[:])
        nc.sync.dma_start(selr_sb[:], d_selr[:])
        if not use_mask:
            bias2f_sb = const.tile([128, C], F16, name="bias2f_sb")
            nc.scalar.copy(bias2f_sb[:], bias2_sb[:])
        if use_bias:
            bq_sb = const.tile([DIM, 1], F32, name="bq_sb")
            bk_sb = const.tile([DIM, 1], F32, name="bk_sb")
            bv2_sb = const.tile([128, C], F32, name="bv2_sb")
            bp2_sb = const.tile([128, C], F32, name="bp2_sb")
            nc.sync.dma_start(bq_sb[:], d_bq[:])
            nc.sync.dma_start(bk_sb[:], d_bk[:])
            nc.sync.dma_start(bv2_sb[:], d_bv2[:])
            nc.sync.dma_start(bp2_sb[:], d_bp2[:])

        # Persistent zero-block tiles (double-buffered by group / pair
        # parity). Data blocks are rewritten every use; zero blocks are
        # memset once here and never touched again.
        thq6 = [const.tile([DIM, HEADS, TOK_G], F16, name=f"thq6_{i}")
                for i in range(2)]
        een2 = [const.tile([128, HEADS, 2, N], F16, name=f"een2_{i}")
                for i in range(2)]
        for i in range(2):
            nc.vector.memset(thq6[i][:].rearrange("p h t -> p (h t)"), 0.0)
            nc.vector.memset(een2[i][:].rearrange("p h s n -> p (h s n)"), 0.0)

        def load_cast_transpose(src_flat, tok0, tag, on_vector):
            """Load 512 tokens natural, cast to f16, PE-transpose."""
            nat = p_nat.tile([128, 4, C], F32, tag=f"nat_{tag}", name=f"nat_{tag}")
            nc.sync.dma_start(
                nat[:],
                src_flat[tok0 : tok0 + TOK_G, :].rearrange("(t p) c -> p t c", p=128),
            )
            n16 = p_n16.tile([128, 4, C], F16, tag=f"n16_{tag}", name=f"n16_{tag}")
            cast_dst = n16[:].rearrange("p t c -> p (t c)")
            cast_src = nat[:].rearrange("p t c -> p (t c)")
            if on_vector:
                nc.vector.tensor_copy(cast_dst, cast_src)
            else:
                nc.scalar.copy(cast_dst, cast_src)
            xt = p_xt.tile([128, 3, TOK_G], F16, tag=f"xt_{tag}", name=f"xt_{tag}")
            for t in range(4):
                tp = ps_g.tile([128, C], F16, tag="psg", name=f"tp_{tag}{t}")
                for Ci in range(3):
                    nc.tensor.transpose(
                        tp[:, 128 * Ci : 128 * (Ci + 1)],
                        n16[:, t, 128 * Ci : 128 * (Ci + 1)],
                        identc[:],
                    )
                dst = xt[:, :, 128 * t : 128 * (t + 1)]
                srcv = tp[:].rearrange("p (c f) -> p c f", c=3)
                if t % 2 == 0:
                    nc.vector.tensor_copy(dst, srcv)
                else:
                    nc.scalar.copy(dst, srcv)
            return xt

        for g in range(n_grp):
            tok0 = g * TOK_G
            xt = load_cast_transpose(x_flat, tok0, "x", True)
            cxt = load_cast_transpose(cx_flat, tok0, "c", False)

            # ---- Q/K projections: kT natural is the QK lhsT directly;
            # q relocated into the zero-block thq6 layout by 6 DMAs ----
            def proj_qk(src_t, w, b, tag):
                ps = ps_g.tile([DIM, TOK_G], F32, tag="psg", name=f"pqk_{tag}")
                for Ci in range(3):
                    nc.tensor.matmul(
                        ps[:],
                        w[:, Ci],
                        src_t[:, Ci],
                        start=(Ci == 0),
                        stop=(Ci == 2),
                    )
                sb = p_qk.tile([DIM, TOK_G], F16, tag=f"qk_{tag}", name=f"qk_{tag}")
                if use_bias:
                    nc.scalar.activation(
                        sb[:], ps[:], mybir.ActivationFunctionType.Identity, bias=b[:]
                    )
                else:
                    nc.scalar.copy(sb[:], ps[:])
                return sb

            kh = proj_qk(cxt, wk_sb, bk_sb if use_bias else None, "k")
            qtmp = proj_qk(xt, wq_sb, bq_sb if use_bias else None, "q")
            qz = thq6[g % 2]
            for h in range(HEADS):
                eng = nc.scalar if h % 2 else nc.sync
                eng.dma_start(
                    qz[HD * h : HD * h + HD, h], qtmp[HD * h : HD * h + HD, :]
                )
            og = p_out.tile([128, 4, C], F32, tag="og", name="og")

            for pp in range(4):
                ptok = 128 * pp  # within-group token base
                # ---- V projection (pair-natural: rows = m tokens) ----
                vps = ps_vpf.tile([128, C], F32, tag="vpf", name="vps")
                for Ci in range(3):
                    nc.tensor.matmul(
                        vps[:],
                        cxt[:, Ci, ptok : ptok + 128],
                        wv_sb[:, Ci],
                        start=(Ci == 0),
                        stop=(Ci == 2),
                    )
                v = p_v.tile([128, C], F16, tag="v", name="v")
                if use_bias:
                    nc.vector.tensor_tensor(
                        v[:], vps[:], bv2_sb[:], op=mybir.AluOpType.add
                    )
                else:
                    nc.vector.tensor_copy(v[:], vps[:])

                # ---- QK -> transposed logits L^T[m, (h, n)]: bias seeded
                # into PSUM by an identity-matmul, then one QK matmul per
                # window (all heads stacked along K=96) accumulates ----
                ltps = ps_l.tile([128, C], F32, tag="lt", name="ltps")
                if not use_mask:
                    nc.tensor.matmul(
                        ltps[:], identc[:], bias2f_sb[:], start=True, stop=False,
                        skip_group_check=True,
                    )
                for s in range(2):
                    tok = ptok + 64 * s
                    nc.tensor.matmul(
                        ltps[64 * s : 64 * s + 64, :],
                        kh[:, tok : tok + 64],
                        qz[:, :, tok : tok + 64],
                        start=use_mask,
                        stop=True,
                        tile_position=(0, 64 * s),
                        skip_group_check=True,
                    )

                # ---- softmax (transposed layout, normalize pre-AV) ----
                eeb = p_sm.tile([128, C], F16, tag="eeb", name="eeb")
                if use_mask:
                    e32 = p_sm.tile([128, C], F32, tag="e32", name="e32")
                    nc.vector.tensor_tensor(
                        e32[:], ltps[:], bias2_sb[:], op=mybir.AluOpType.add
                    )
                    m_sb = p_sm.tile([128, C], F32, tag="msk", name="m_sb")
                    nc.sync.dma_start(m_sb[:], d_mask2[g * 4 + pp])
                    nc.vector.tensor_tensor(
                        e32[:], e32[:], m_sb[:], op=mybir.AluOpType.add
                    )
                    nc.scalar.activation(
                        eeb[:], e32[:], mybir.ActivationFunctionType.Exp
                    )
                else:
                    nc.scalar.activation(
                        eeb[:], ltps[:], mybir.ActivationFunctionType.Exp
                    )
                # sums replicated to all partitions in one matmul:
                # selr[m, j] = 1 iff block(m) == block(j), so
                # srep[j, (h, n)] = sum_{m in block(j)} eeb[m, (h, n)].
                srep = ps_l.tile([128, C], F32, tag="lt", name="srep")
                nc.tensor.matmul(srep[:], selr_sb[:], eeb[:], start=True, stop=True)
                recf = p_sm.tile([128, C], F32, tag="rec", name="recf")
                nc.vector.reciprocal_approx_fast(recf[:], srep[:])
                recf16 = p_sm.tile([128, C], F16, tag="rec16", name="recf16")
                nc.vector.tensor_copy(recf16[:], recf[:])
                # normalize into the zero-block een2 layout [m, (h, s, n)]:
                # window A rows fill the s=0 blocks, window B rows the
                # s=1 blocks; the off-window blocks stay zero.
                ee2 = een2[pp % 2]
                nc.vector.tensor_tensor(
                    ee2[0:64, :, 0, :],
                    eeb[0:64, :].rearrange("p (h n) -> p h n", h=HEADS),
                    recf16[0:64, :].rearrange("p (h n) -> p h n", h=HEADS),
                    op=mybir.AluOpType.mult,
                )
                nc.vector.tensor_tensor(
                    ee2[64:128, :, 1, :],
                    eeb[64:128, :].rearrange("p (h n) -> p h n", h=HEADS),
                    recf16[64:128, :].rearrange("p (h n) -> p h n", h=HEADS),
                    op=mybir.AluOpType.mult,
                )

                # ---- AV: one matmul per head (K=128 = both windows' m,
                # v pair-natural), col-packed by head parity ----
                pps = ps_vpf.tile([128, C], F32, tag="vpf", name="pps")
                for Ci in range(3):
                    for a in range(2):
                        h = 2 * Ci + a
                        nc.tensor.matmul(
                            pps[64 * a : 64 * a + 64,
                                128 * Ci : 128 * Ci + 128],
                            v[:, 64 * h : 64 * h + 64],
                            ee2[:, h].rearrange("p s n -> p (s n)"),
                            start=True,
                            stop=True,
                            tile_position=(0, 64 * a),
                        )
                pl = p_pl.tile([128, C], F16, tag="pl", name="pl")
                nc.scalar.copy(pl[:], pps[:])

                # ---- output projection ----
                fps = ps_vpf.tile([128, C], F32, tag="vpf", name="fps")
                for Ci in range(3):
                    nc.tensor.matmul(
                        fps[:],
                        pl[:, 128 * Ci : 128 * (Ci + 1)],
                        wp_sb[:, Ci],
                        start=(Ci == 0),
                        stop=(Ci == 2),
                    )
                if use_bias:
                    nc.vector.tensor_tensor(
                        og[:, pp], fps[:], bp2_sb[:], op=mybir.AluOpType.add
                    )
                else:
                    nc.scalar.copy(og[:, pp], fps[:])
            nc.scalar.dma_start(
                y_flat[tok0 : tok0 + TOK_G, :].rearrange("(t p) c -> p t c", p=128),
                og[:],
            )

    nc.compile()
    return nc


def _prep_host(Wq, bq, Wk, bk, Wv, bv, Wp, bp, rpi, rpb_table, mask):
    scale = HD ** (-0.5)
    Wq = np.asarray(Wq, dtype=np.float32) * scale
    bq = np.asarray(bq, dtype=np.float32) * scale
    Wk = np.asarray(Wk, dtype=np.float32)
    bk = np.asarray(bk, dtype=np.float32)

    tbl = np.asarray(rpb_table, dtype=np.float32)
    rp = np.asarray(rpi).astype(np.int64)
    bias_nmh = tbl[rp.reshape(-1)].reshape(N, N, HEADS)  # (n, m, h)
    b_mhn = bias_nmh.transpose(1, 2, 0).reshape(N, C)  # (m, (h, n))
    bias2t = np.concatenate([b_mhn, b_mhn], axis=0).astype(np.float32)  # (128, C)

    selr = np.zeros((128, 128), np.float16)
    selr[0:64, 0:64] = 1.0
    selr[64:128, 64:128] = 1.0

    bv2 = np.tile(np.asarray(bv, dtype=np.float32)[None, :], (128, 1))
    bp2 = np.tile(np.asarray(bp, dtype=np.float32)[None, :], (128, 1))

    consts = {
        "wq": Wq.astype(np.float16),
        "wk": Wk.astype(np.float16),
        "wv": np.asarray(Wv, dtype=np.float32).astype(np.float16),
        "wp": np.asarray(Wp, dtype=np.float32).astype(np.float16),
        "bias2t": bias2t,
        "selr": selr,
    }
    use_bias = bool(
        np.any(bq) or np.any(bk) or np.any(np.asarray(bv)) or np.any(np.asarray(bp))
    )
    if use_bias:
        consts.update({"bq_c": bq.reshape(DIM, 1).copy(),
                       "bk_c": bk.reshape(DIM, 1).copy(),
                       "bv2": bv2, "bp2": bp2})

    mask = np.asarray(mask, dtype=np.float32)
    use_mask = bool(np.any(mask))
    return consts, use_bias, use_mask, mask


def _mask2_for_core(mask, w0, nw):
    """(nw//2, 128, 384): rows = pair m-tokens, cols = (h, n) tiled."""
    nwin = mask.shape[0]
    out = np.empty((nw // 2, 128, C), dtype=np.float32)
    for p in range(nw // 2):
        wa = (w0 + 2 * p) % nwin
        wb = (w0 + 2 * p + 1) % nwin
        blk = np.concatenate([mask[wa].T, mask[wb].T], axis=0)  # (128, 64) [m, n]
        out[p] = np.tile(blk, (1, HEADS))
    return out


_CACHE = {}


def prepare(x, cross_x, rpi, mask, Wq, bq, Wk, bk, Wv, bv, Wp, bp, rpb_table):
    """Host prep + module build; returns (nc, in_maps)."""
    x = np.ascontiguousarray(np.asarray(x, dtype=np.float32))
    cross_x = np.ascontiguousarray(np.asarray(cross_x, dtype=np.float32))
    b_ = x.shape[0]
    assert b_ % NCORES == 0
    nw = b_ // NCORES

    consts, use_bias, use_mask, mask_f = _prep_host(
        Wq, bq, Wk, bk, Wv, bv, Wp, bp, rpi, rpb_table, mask
    )

    key = (nw, use_mask, use_bias)
    if key not in _CACHE:
        _CACHE[key] = _build(nw, use_mask, use_bias)
    nc = _CACHE[key]

    in_maps = []
    for i in range(NCORES):
        m = dict(consts)
        m["x"] = x[i * nw : (i + 1) * nw]
        m["cx"] = cross_x[i * nw : (i + 1) * nw]
        if use_mask:
            m["mask2t"] = _mask2_for_core(mask_f, i * nw, nw)
        in_maps.append(m)
    return nc, in_maps


def kernel(x, cross_x, rpi, mask, Wq, bq, Wk, bk, Wv, bv, Wp, bp, rpb_table):
    nc, in_maps = prepare(
        x, cross_x, rpi, mask, Wq, bq, Wk, bk, Wv, bv, Wp, bp, rpb_table
    )
    res = run_bass_kernel_spmd(
        nc,
        in_maps,
        core_ids=list(range(NCORES)),
        trace=bool(int(os.environ.get("KERNEL_TRACE", "0"))),
    )
    out = np.concatenate([res.results[i]["y"] for i in range(NCORES)], axis=0)
    kernel.last_exec_time_ns = res.exec_time_ns
    return out


kernel.last_exec_time_ns = None


# revision 67
# speedup vs baseline: 1.0099x; 1.0099x over previous
"""Trainium2 Bass kernel for CrossBandWindowAttention.

Reference computation (per window item b of B_=2048):
    q = (x @ Wq + bq) * scale      -> (64, 96), 6 heads x 16
    k = cross_x @ Wk + bk          -> (64, 96)
    v = cross_x @ Wv + bv          -> (64, 384), 6 heads x 64
    L_h = q_h k_h^T + rpb_bias_h (+ mask_w)
    A = softmax(L, axis=-1)
    out = (concat_h A_h v_h) @ Wp + bp

Sharding: data-parallel over b_ across 8 cores (256 windows each).
Weights / bias table replicated; rpi+rpb_table folded on host into a
(128, 384) additive bias tile in TRANSPOSED layout (rows = two windows'
64 m-tokens, cols = (h, n)).

Per-core design (all PE operands f16 -> full-rate streaming + FWL).
NOTE: this runtime crashes on >2 back-to-back matmuls whose
tile_position row strips differ, so everything uses row-0 tiles with at
most 2-way column packing (the baseline-proven constructs). Instruction
count is cut by K-dim stacking with zero-padded moving operands instead:
  - x/cross_x loaded natural (tokens on partitions), cast to f16 on
    DVE/ACT, then PE-transposed (f16, 1 cyc/row) to xt/cxt
    (channels on partitions, tokens free).
  - QK computes the TRANSPOSED logits L^T[m, (h, n)] for one window in
    ONE matmul: lhsT = kT natural [96=(h,d), 64 m-toks] (all heads
    stacked along K), rhs = th_q6 [96, (h, n)] where block h holds q_h
    at rows 16h..16h+16 and ZEROS elsewhere -- the zeros kill cross-head
    terms, and PE time only depends on streamed columns (384).
    th_q6 is filled by 6 SBUF-SBUF DMAs per group into persistent
    zero-initialized double buffers.
  - softmax: exp on ACT; multiply by exp(bias^T); per-(h, n) sums via a
    ones-matmul (lhsT = block-indicator [128, 2]); reciprocal on DVE;
    broadcast back over partitions via a K=2 matmul; normalize on DVE
    into een2 [128, (h, s, n)] with the same zero-block trick.
    No probability transposes, no max-subtraction (logits are O(1)).
  - AV: one matmul per head (K=128 = both windows' m stacked, v stays
    pair-natural -- no splits), N=128 = (s, n), col-packed in head
    pairs, into the proj-lhsT tile pl[(h%2)*64+vd, (Ci, s, n)].
  - V / final projections: 3 accumulating matmuls each per pair.
PSUM plan (8 banks): transposes+qkT pool 2, logits^T+sums pool 2,
v/av/final rotating pool 4.
"""

import os
from contextlib import ExitStack

import numpy as np

import concourse.bass as bass
import concourse.mybir as mybir
import concourse.tile as tile
from concourse import bacc
from concourse.bass_utils import run_bass_kernel_spmd
from concourse.masks import make_identity

F32 = mybir.dt.float32
F32R = mybir.dt.float32r
F16 = mybir.dt.float16

DIM = 96
HEADS = 6
HD = 16  # head dim for q/k
VD = 64  # head dim for v
N = 64  # tokens per window
C = 384
NCORES = 8
B_TOTAL = 2048
NW_CORE = B_TOTAL // NCORES  # 256 windows per core
GRP = 8  # windows per group (512 tokens)
TOK_G = GRP * N  # 512


def _build(nw, use_mask, use_bias):
    """Build the per-core Bass module for `nw` windows."""
    nc = bacc.Bacc("TRN2", target_bir_lowering=False, debug=False)

    d_x = nc.dram_tensor("x", [nw, N, C], F32, kind="ExternalInput").ap()
    d_cx = nc.dram_tensor("cx", [nw, N, C], F32, kind="ExternalInput").ap()
    d_wq = nc.dram_tensor("wq", [C, DIM], F16, kind="ExternalInput").ap()
    d_wk = nc.dram_tensor("wk", [C, DIM], F16, kind="ExternalInput").ap()
    d_wv = nc.dram_tensor("wv", [C, C], F16, kind="ExternalInput").ap()
    d_wp = nc.dram_tensor("wp", [C, C], F16, kind="ExternalInput").ap()
    d_bias2 = nc.dram_tensor("bias2t", [128, C], F32, kind="ExternalInput").ap()
    d_selr = nc.dram_tensor("selr", [128, 128], F16, kind="ExternalInput").ap()
    if use_bias:
        d_bq = nc.dram_tensor("bq_c", [DIM, 1], F32, kind="ExternalInput").ap()
        d_bk = nc.dram_tensor("bk_c", [DIM, 1], F32, kind="ExternalInput").ap()
        d_bv2 = nc.dram_tensor("bv2", [128, C], F32, kind="ExternalInput").ap()
        d_bp2 = nc.dram_tensor("bp2", [128, C], F32, kind="ExternalInput").ap()
    if use_mask:
        d_mask2 = nc.dram_tensor(
            "mask2t", [nw // 2, 128, C], F32, kind="ExternalInput"
        ).ap()
    d_y = nc.dram_tensor("y", [nw, N, C], F32, kind="ExternalOutput").ap()

    x_flat = d_x.rearrange("w n c -> (w n) c")
    cx_flat = d_cx.rearrange("w n c -> (w n) c")
    y_flat = d_y.rearrange("w n c -> (w n) c")

    n_grp = nw // GRP

    with tile.TileContext(nc) as tc, ExitStack() as ctx:
        const = ctx.enter_context(tc.tile_pool(name="const", bufs=1))
        p_nat = ctx.enter_context(tc.tile_pool(name="p_nat", bufs=2))
        p_n16 = ctx.enter_context(tc.tile_pool(name="p_n16", bufs=2))
        p_xt = ctx.enter_context(tc.tile_pool(name="p_xt", bufs=2))
        p_qk = ctx.enter_context(tc.tile_pool(name="p_qk", bufs=2))
        p_v = ctx.enter_context(tc.tile_pool(name="p_v", bufs=2))
        p_sm = ctx.enter_context(tc.tile_pool(name="p_sm", bufs=2))
        p_pl = ctx.enter_context(tc.tile_pool(name="p_pl", bufs=2))
        p_out = ctx.enter_context(tc.tile_pool(name="p_out", bufs=2))
        # PSUM: 8 banks total.
        ps_g = ctx.enter_context(tc.tile_pool(name="ps_g", bufs=2, space="PSUM"))
        ps_l = ctx.enter_context(tc.tile_pool(name="ps_l", bufs=2, space="PSUM"))
        ps_vpf = ctx.enter_context(tc.tile_pool(name="ps_vpf", bufs=4, space="PSUM"))

        # ---- constants in SBUF ----
        identc = const.tile([128, 128], F16, name="identc")
        make_identity(nc, identc[:])

        wq_sb = const.tile([128, 3, DIM], F16, name="wq_sb")
        wk_sb = const.tile([128, 3, DIM], F16, name="wk_sb")
        wv_sb = const.tile([128, 3, C], F16, name="wv_sb")
        wp_sb = const.tile([128, 3, C], F16, name="wp_sb")
        bias2_sb = const.tile([128, C], F32, name="bias2_sb")
        selr_sb = const.tile([128, 128], F16, name="selr_sb")
        for Ci in range(3):
            sl = slice(128 * Ci, 128 * Ci + 128)
            nc.sync.dma_start(wq_sb[:, Ci], d_wq[sl, :])
            nc.sync.dma_start(wk_sb[:, Ci], d_wk[sl, :])
            nc.sync.dma_start(wv_sb[:, Ci], d_wv[sl, :])
            nc.sync.dma_start(wp_sb[:, Ci], d_wp[sl, :])
        nc.sync.dma_start(bias2_sb[:], d_bias2[:])
        nc.sync.dma_start(selr_sb[:], d_selr[:])
        if not use_mask:
            bias2f_sb = const.tile([128, C], F16, name="bias2f_sb")
            nc.scalar.copy(bias2f_sb[:], bias2_sb[:])
        if use_bias:
            bq_sb = const.tile([DIM, 1], F32, name="bq_sb")
            bk_sb = const.tile([DIM, 1], F32, name="bk_sb")
            bv2_sb = const.tile([128, C], F32, name="bv2_sb")
            bp2_sb = const.tile([128, C], F32, name="bp2_sb")
            nc.sync.dma_start(bq_sb[:], d_bq[:])
            nc.sync.dma_start(bk_sb[:], d_bk[:])
            nc.sync.dma_start(bv2_sb[:], d_bv2[:])
            nc.sync.dma_start(bp2_sb[:], d_bp2[:])

        # Persistent zero-block tiles (double-buffered by group / pair
        # parity). Data blocks are rewritten every use; zero blocks are
        # memset once here and never touched again.
        thq6 = [const.tile([DIM, HEADS, TOK_G], F16, name=f"thq6_{i}")
                for i in range(2)]
        een2 = [const.tile([128, HEADS, 2, N], F16, name=f"een2_{i}")
                for i in range(2)]
        for i in range(2):
            nc.vector.memset(thq6[i][:].rearrange("p h t -> p (h t)"), 0.0)
            nc.vector.memset(een2[i][:].rearrange("p h s n -> p (h s n)"), 0.0)

        def load_cast_transpose(src_flat, tok0, tag, on_vector):
            """Load 512 tokens natural, cast to f16, PE-transpose."""
            nat = p_nat.tile([128, 4, C], F32, tag=f"nat_{tag}", name=f"nat_{tag}")
            nc.sync.dma_start(
                nat[:],
                src_flat[tok0 : tok0 + TOK_G, :].rearrange("(t p) c -> p t c", p=128),
            )
            n16 = p_n16.tile([128, 4, C], F16, tag=f"n16_{tag}", name=f"n16_{tag}")
            cast_dst = n16[:].rearrange("p t c -> p (t c)")
            cast_src = nat[:].rearrange("p t c -> p (t c)")
            if on_vector:
                nc.vector.tensor_copy(cast_dst, cast_src)
            else:
                nc.scalar.copy(cast_dst, cast_src)
            xt = p_xt.tile([128, 3, TOK_G], F16, tag=f"xt_{tag}", name=f"xt_{tag}")
            for t in range(4):
                tp = ps_g.tile([128, C], F16, tag="psg", name=f"tp_{tag}{t}")
                for Ci in range(3):
                    nc.tensor.transpose(
                        tp[:, 128 * Ci : 128 * (Ci + 1)],
                        n16[:, t, 128 * Ci : 128 * (Ci + 1)],
                        identc[:],
                    )
                dst = xt[:, :, 128 * t : 128 * (t + 1)]
                srcv = tp[:].rearrange("p (c f) -> p c f", c=3)
                if t % 2 == 0:
                    nc.vector.tensor_copy(dst, srcv)
                else:
                    nc.scalar.copy(dst, srcv)
            return xt

        for g in range(n_grp):
            tok0 = g * TOK_G
            xt = load_cast_transpose(x_flat, tok0, "x", True)
            cxt = load_cast_transpose(cx_flat, tok0, "c", False)

            # ---- Q/K projections: kT natural is the QK lhsT directly;
            # q relocated into the zero-block thq6 layout by 6 DMAs ----
            def proj_qk(src_t, w, b, tag):
                ps = ps_g.tile([DIM, TOK_G], F32, tag="psg", name=f"pqk_{tag}")
                for Ci in range(3):
                    nc.tensor.matmul(
                        ps[:],
                        w[:, Ci],
                        src_t[:, Ci],
                        start=(Ci == 0),
                        stop=(Ci == 2),
                    )
                sb = p_qk.tile([DIM, TOK_G], F16, tag=f"qk_{tag}", name=f"qk_{tag}")
                if use_bias:
                    nc.scalar.activation(
                        sb[:], ps[:], mybir.ActivationFunctionType.Identity, bias=b[:]
                    )
                else:
                    nc.scalar.copy(sb[:], ps[:])
                return sb

            kh = proj_qk(cxt, wk_sb, bk_sb if use_bias else None, "k")
            qtmp = proj_qk(xt, wq_sb, bq_sb if use_bias else None, "q")
            qz = thq6[g % 2]
            for h in range(HEADS):
                eng = nc.scalar if h % 2 else nc.sync
                eng.dma_start(
                    qz[HD * h : HD * h + HD, h], qtmp[HD * h : HD * h + HD, :]
                )
            og = p_out.tile([128, 4, C], F32, tag="og", name="og")

            for pp in range(4):
                ptok = 128 * pp  # within-group token base
                # ---- V projection (pair-natural: rows = m tokens) ----
                vps = ps_vpf.tile([128, C], F32, tag="vpf", name="vps")
                for Ci in range(3):
                    nc.tensor.matmul(
                        vps[:],
                        cxt[:, Ci, ptok : ptok + 128],
                        wv_sb[:, Ci],
                        start=(Ci == 0),
                        stop=(Ci == 2),
                    )
                v = p_v.tile([128, C], F16, tag="v", name="v")
                if use_bias:
                    nc.vector.tensor_tensor(
                        v[:], vps[:], bv2_sb[:], op=mybir.AluOpType.add
                    )
                else:
                    nc.vector.tensor_copy(v[:], vps[:])

                # ---- QK -> transposed logits L^T[m, (h, n)]: bias seeded
                # into PSUM by an identity-matmul, then one QK matmul per
                # window (all heads stacked along K=96) accumulates ----
                ltps = ps_l.tile([128, C], F32, tag="lt", name="ltps")
                if not use_mask:
                    nc.tensor.matmul(
                        ltps[:], identc[:], bias2f_sb[:], start=True, stop=False,
                        skip_group_check=True,
                    )
                for s in range(2):
                    tok = ptok + 64 * s
                    nc.tensor.matmul(
                        ltps[64 * s : 64 * s + 64, :],
                        kh[:, tok : tok + 64],
                        qz[:, :, tok : tok + 64],
                        start=use_mask,
                        stop=True,
                        tile_position=(0, 64 * s),
                        skip_group_check=True,
                    )

                # ---- softmax (transposed layout, normalize pre-AV) ----
                eeb = p_sm.tile([128, C], F16, tag="eeb", name="eeb")
                if use_mask:
                    e32 = p_sm.tile([128, C], F32, tag="e32", name="e32")
                    nc.vector.tensor_tensor(
                        e32[:], ltps[:], bias2_sb[:], op=mybir.AluOpType.add
                    )
                    m_sb = p_sm.tile([128, C], F32, tag="msk", name="m_sb")
                    nc.sync.dma_start(m_sb[:], d_mask2[g * 4 + pp])
                    nc.vector.tensor_tensor(
                        e32[:], e32[:], m_sb[:], op=mybir.AluOpType.add
                    )
                    nc.scalar.activation(
                        eeb[:], e32[:], mybir.ActivationFunctionType.Exp
                    )
                else:
                    nc.scalar.activation(
                        eeb[:], ltps[:], mybir.ActivationFunctionType.Exp
                    )
                # sums replicated to all partitions in one matmul:
                # selr[m, j] = 1 iff block(m) == block(j), so
                # srep[j, (h, n)] = sum_{m in block(j)} eeb[m, (h, n)].
                srep = ps_l.tile([128, C], F32, tag="lt", name="srep")
                nc.tensor.matmul(srep[:], selr_sb[:], eeb[:], start=True, stop=True)
                recf = p_sm.tile([128, C], F32, tag="rec", name="recf")
                nc.vector.reciprocal_approx_fast(recf[:], srep[:])
                # normalize into the zero-block een2 layout [m, (h, s, n)]:
                # window A rows fill the s=0 blocks, window B rows the
                # s=1 blocks; the off-window blocks stay zero.
                ee2 = een2[pp % 2]
                nc.vector.tensor_tensor(
                    ee2[0:64, :, 0, :],
                    eeb[0:64, :].rearrange("p (h n) -> p h n", h=HEADS),
                    recf[0:64, :].rearrange("p (h n) -> p h n", h=HEADS),
                    op=mybir.AluOpType.mult,
                )
                nc.vector.tensor_tensor(
                    ee2[64:128, :, 1, :],
                    eeb[64:128, :].rearrange("p (h n) -> p h n", h=HEADS),
                    recf[64:128, :].rearrange("p (h n) -> p h n", h=HEADS),
                    op=mybir.AluOpType.mult,
                )

                # ---- AV: one matmul per head (K=128 = both windows' m,
                # v pair-natural), col-packed by head parity ----
                pps = ps_vpf.tile([128, C], F32, tag="vpf", name="pps")
                for Ci in range(3):
                    for a in range(2):
                        h = 2 * Ci + a
                        nc.tensor.matmul(
                            pps[64 * a : 64 * a + 64,
                                128 * Ci : 128 * Ci + 128],
                            v[:, 64 * h : 64 * h + 64],
                            ee2[:, h].rearrange("p s n -> p (s n)"),
                            start=True,
                            stop=True,
                            tile_position=(0, 64 * a),
                        )
                pl = p_pl.tile([128, C], F16, tag="pl", name="pl")
                nc.scalar.copy(pl[:], pps[:])

                # ---- output projection ----
                fps = ps_vpf.tile([128, C], F32, tag="vpf", name="fps")
                for Ci in range(3):
                    nc.tensor.matmul(
                        fps[:],
                        pl[:, 128 * Ci : 128 * (Ci + 1)],
                        wp_sb[:, Ci],
                        start=(Ci == 0),
                        stop=(Ci == 2),
                    )
                if use_bias:
                    nc.vector.tensor_tensor(
                        og[:, pp], fps[:], bp2_sb[:], op=mybir.AluOpType.add
                    )
                else:
                    nc.scalar.copy(og[:, pp], fps[:])
            nc.scalar.dma_start(
                y_flat[tok0 : tok0 + TOK_G, :].rearrange("(t p) c -> p t c", p=128),
                og[:],
            )

    nc.compile()
    return nc


def _prep_host(Wq, bq, Wk, bk, Wv, bv, Wp, bp, rpi, rpb_table, mask):
    scale = HD ** (-0.5)
    Wq = np.asarray(Wq, dtype=np.float32) * scale
    bq = np.asarray(bq, dtype=np.float32) * scale
    Wk = np.asarray(Wk, dtype=np.float32)
    bk = np.asarray(bk, dtype=np.float32)

    tbl = np.asarray(rpb_table, dtype=np.float32)
    rp = np.asarray(rpi).astype(np.int64)
    bias_nmh = tbl[rp.reshape(-1)].reshape(N, N, HEADS)  # (n, m, h)
    b_mhn = bias_nmh.transpose(1, 2, 0).reshape(N, C)  # (m, (h, n))
    bias2t = np.concatenate([b_mhn, b_mhn], axis=0).astype(np.float32)  # (128, C)

    selr = np.zeros((128, 128), np.float16)
    selr[0:64, 0:64] = 1.0
    selr[64:128, 64:128] = 1.0

    bv2 = np.tile(np.asarray(bv, dtype=np.float32)[None, :], (128, 1))
    bp2 = np.tile(np.asarray(bp, dtype=np.float32)[None, :], (128, 1))

    consts = {
        "wq": Wq.astype(np.float16),
        "wk": Wk.astype(np.float16),
        "wv": np.asarray(Wv, dtype=np.float32).astype(np.float16),
        "wp": np.asarray(Wp, dtype=np.float32).astype(np.float16),
        "bias2t": bias2t,
        "selr": selr,
    }
    use_bias = bool(
        np.any(bq) or np.any(bk) or np.any(np.asarray(bv)) or np.any(np.asarray(bp))
    )
    if use_bias:
        consts.update({"bq_c": bq.reshape(DIM, 1).copy(),
                       "bk_c": bk.reshape(DIM, 1).copy(),
                       "bv2": bv2, "bp2": bp2})

    mask = np.asarray(mask, dtype=np.float32)
    use_mask = bool(np.any(mask))
    return consts, use_bias, use_mask, mask


def _mask2_for_core(mask, w0, nw):
    """(nw//2, 128, 384): rows = pair m-tokens, cols = (h, n) tiled."""
    nwin = mask.shape[0]
    out = np.empty((nw // 2, 128, C), dtype=np.float32)
    for p in range(nw // 2):
        wa = (w0 + 2 * p) % nwin
        wb = (w0 + 2 * p + 1) % nwin
        blk = np.concatenate([mask[wa].T, mask[wb].T], axis=0)  # (128, 64) [m, n]
        out[p] = np.tile(blk, (1, HEADS))
    return out


_CACHE = {}


def prepare(x, cross_x, rpi, mask, Wq, bq, Wk, bk, Wv, bv, Wp, bp, rpb_table):
    """Host prep + module build; returns (nc, in_maps)."""
    x = np.ascontiguousarray(np.asarray(x, dtype=np.float32))
    cross_x = np.ascontiguousarray(np.asarray(cross_x, dtype=np.float32))
    b_ = x.shape[0]
    assert b_ % NCORES == 0
    nw = b_ // NCORES

    consts, use_bias, use_mask, mask_f = _prep_host(
        Wq, bq, Wk, bk, Wv, bv, Wp, bp, rpi, rpb_table, mask
    )

    key = (nw, use_mask, use_bias)
    if key not in _CACHE:
        _CACHE[key] = _build(nw, use_mask, use_bias)
    nc = _CACHE[key]

    in_maps = []
    for i in range(NCORES):
        m = dict(consts)
        m["x"] = x[i * nw : (i + 1) * nw]
        m["cx"] = cross_x[i * nw : (i + 1) * nw]
        if use_mask:
            m["mask2t"] = _mask2_for_core(mask_f, i * nw, nw)
        in_maps.append(m)
    return nc, in_maps


def kernel(x, cross_x, rpi, mask, Wq, bq, Wk, bk, Wv, bv, Wp, bp, rpb_table):
    nc, in_maps = prepare(
        x, cross_x, rpi, mask, Wq, bq, Wk, bk, Wv, bv, Wp, bp, rpb_table
    )
    res = run_bass_kernel_spmd(
        nc,
        in_maps,
        core_ids=list(range(NCORES)),
        trace=bool(int(os.environ.get("KERNEL_TRACE", "0"))),
    )
    out = np.concatenate([res.results[i]["y"] for i in range(NCORES)], axis=0)
    kernel.last_exec_time_ns = res.exec_time_ns
    return out


kernel.last_exec_time_ns = None
